# revision 14
# baseline (speedup 1.0000x reference)
"""Trainium2 Bass kernel for nn_CBContrastiveLoss (class-balanced focal contrastive loss).

Strategy (8-core SPMD, one compiled NEFF, per-core differences only via inputs):
  - Interleaved data-parallel sharding over samples i: core r owns rows i = r::8.
  - On each core, compute sim TRANSPOSED: tiles [j=128 partitions, i=1024 free],
    so per-row(i) reductions over j become PE matmuls with a one-hot lhsT.
  - Focal loss decomposition (validated vs reference, rel err < 1e-6):
      per_pair = (y - logS)(1 - p)^2,  y = (dot-1)/T,  p = exp(y - logS)
      sum_pos per_pair = T0 - 2*U1 + U2 (U2 dropped, ~3e-7 rel)
      T0 = (fn_i . g_c - 1 - npos)/T - npos*logS   (analytic via class sums g_c)
      U1 = (Q1 - logS*R1)/S
      R1 = sum_pos E, Q1 = sum_pos y*E, S = sum_{j != i} E = sum_c R1[c]
  - Diagonal E_ii zeroed exactly via a tiny per-core [128,16] mask input
    (interleaved sharding makes the diag position core-independent per j-tile).
"""

import numpy as np
import ml_dtypes

import concourse.bass as bass
import concourse.bacc as bacc
import concourse.tile as tile
from concourse import mybir
from concourse.bass_utils import run_bass_kernel_spmd
from concourse.masks import make_identity
from concourse import bass_isa

F32 = mybir.dt.float32
BF16 = mybir.dt.bfloat16

TEMP = 0.07
INV_T = 1.0 / TEMP

N_TOTAL = 8192
D = 512
N_CORES = 8
N_CLS = 9


def build_nc(n_total=N_TOTAL, n_cores=N_CORES, d=D, debug_out=False):
    nshard = n_total // n_cores          # i per core (free dim)
    njt = n_total // 128                 # j tiles
    nkt = d // 128                       # contraction tiles
    win = 128 // n_cores                 # diag window cols per j-tile (16)
    nh = (nshard + 511) // 512           # number of 512-wide N chunks
    ncw = [min(512, nshard - 512 * h) for h in range(nh)]

    nc = bacc.Bacc("TRN2")

    feats = nc.dram_tensor("feats", [n_total, d], F32, kind="ExternalInput")
    fshard = nc.dram_tensor("fshard", [nshard, d], F32, kind="ExternalInput")
    oh9 = nc.dram_tensor("oh9", [njt, 128, N_CLS], BF16, kind="ExternalInput")
    mask16 = nc.dram_tensor("mask16", [128, win], BF16, kind="ExternalInput")
    ohsel = nc.dram_tensor("ohsel", [N_CLS, nshard], BF16, kind="ExternalInput")
    wvn = nc.dram_tensor("wvn", [128, 2, nshard // 128], F32, kind="ExternalInput")
    out = nc.dram_tensor("partial", [1, 1], F32, kind="ExternalOutput")
    if debug_out:
        dbg_sel = nc.dram_tensor("dbg_sel", [128, 4, nshard // 128], F32,
                                 kind="ExternalOutput")
        dbg_R1 = nc.dram_tensor("dbg_R1", [N_CLS, nshard], F32,
                                kind="ExternalOutput")
        dbg_G0 = nc.dram_tensor("dbg_G0", [N_CLS, nshard], F32,
                                kind="ExternalOutput")

    nst = nshard // 128                  # shard row tiles

    with tile.TileContext(nc) as tc:
        with (
            tc.tile_pool(name="consts", bufs=1) as consts,
            tc.tile_pool(name="fnt", bufs=1) as fnt_pool,
            tc.tile_pool(name="pre", bufs=12) as pre,
            tc.tile_pool(name="pre2", bufs=4) as pre2,
            tc.tile_pool(name="grp", bufs=3) as grp,
            tc.tile_pool(name="main", bufs=3) as main,
            tc.tile_pool(name="tail", bufs=1) as tailp,
            tc.tile_pool(name="psA", bufs=2, space="PSUM") as psA,
            tc.tile_pool(name="psAcc", bufs=1, space="PSUM") as psAcc,
        ):
            # ---- constants ----
            oh_sb = consts.tile([128, njt, N_CLS], BF16)
            nc.sync.dma_start(oh_sb, oh9[:].rearrange("t p c -> p t c"))
            mask_sb = consts.tile([128, win], BF16)
            nc.sync.dma_start(mask_sb, mask16[:])
            ohsel_sb = consts.tile([N_CLS, nshard], BF16)
            nc.sync.dma_start(ohsel_sb, ohsel[:])
            wvn_sb = consts.tile([128, 2, nshard // 128], F32)
            nc.sync.dma_start(wvn_sb, wvn[:])
            ident = consts.tile([128, 128], F32)
            make_identity(nc, ident)
            ones9 = consts.tile([N_CLS, 1], F32)
            nc.vector.memset(ones9, 1.0)
            ones128 = consts.tile([128, 1], F32)
            nc.vector.memset(ones128, 1.0)
            zero_b = consts.tile([128, 1], F32)
            nc.vector.memset(zero_b, 0.0)
            negit_b = consts.tile([128, 1], F32)
            nc.vector.memset(negit_b, -INV_T)
            # warmup activation: absorbs the ACT table-load wait (walrus
            # attaches it to the first ACTIVATE, which then allows only one
            # user wait)
            warm = consts.tile([128, 1], F32)
            nc.scalar.activation(warm, zero_b,
                                 mybir.ActivationFunctionType.Square,
                                 bias=zero_b)

            fnT = fnt_pool.tile([128, nkt, n_total], BF16)   # full, transposed
            fnTs = fnt_pool.tile([128, nkt, nshard], BF16)   # shard, transposed

            g_ps = psAcc.tile([N_CLS, d], F32, tag="R1")

            # ---- preamble: normalize features, build fnT + class sums g ----
            def norm_tiles(src_dram, ntiles, dst_fnT, with_g):
                group = 8
                for t0 in range(0, ntiles, group):
                    gn = min(group, ntiles - t0)
                    n2g = grp.tile([128, group], F32, tag="n2")
                    rng = grp.tile([128, group], F32, tag="rn")
                    fts = []
                    for t in range(t0, t0 + gn):
                        ft = pre.tile([128, d], F32, tag="ft")
                        nc.sync.dma_start(ft, src_dram[t * 128:(t + 1) * 128, :])
                        sq = pre2.tile([128, d], F32, tag="sq")
                        nc.scalar.activation(
                            sq, ft, mybir.ActivationFunctionType.Square,
                            bias=zero_b,
                            accum_out=n2g[:, t - t0:t - t0 + 1],
                        )
                        fts.append(ft)
                    # rn = exp(-0.5 * ln(n2))  (avoids inaccurate Rsqrt table)
                    nc.scalar.activation(rng[:, 0:gn], n2g[:, 0:gn],
                                         mybir.ActivationFunctionType.Ln,
                                         bias=zero_b)
                    nc.scalar.activation(rng[:, 0:gn], rng[:, 0:gn],
                                         mybir.ActivationFunctionType.Exp,
                                         bias=zero_b, scale=-0.5)
                    for t in range(t0, t0 + gn):
                        ft = fts[t - t0]
                        fnb = pre2.tile([128, d], BF16, tag="fnb")
                        nc.vector.tensor_scalar_mul(
                            out=fnb, in0=ft, scalar1=rng[:, t - t0:t - t0 + 1])
                        if with_g:
                            nc.tensor.matmul(
                                g_ps, oh_sb[:, t, :], fnb,
                                start=(t == 0), stop=(t == ntiles - 1))
                        for k in range(nkt):
                            nc.sync.dma_start_transpose(
                                dst_fnT[:, k, t * 128:(t + 1) * 128],
                                fnb[:, k * 128:(k + 1) * 128])

            norm_tiles(feats, njt, fnT, with_g=True)
            norm_tiles(fshard, nst, fnTs, with_g=False)

            # ---- g -> gT (bf16) ; G0[c,i] = fn_i . g_c ----
            g_sb = tailp.tile([N_CLS, d], F32)
            nc.scalar.copy(g_sb, g_ps)
            gT_sb = tailp.tile([128, nkt, N_CLS], BF16)
            for k in range(nkt):
                gtp = psA.tile([128, N_CLS], F32, tag="z")
                nc.tensor.transpose(gtp, g_sb[0:N_CLS, k * 128:(k + 1) * 128],
                                    ident[0:N_CLS, 0:N_CLS])
                nc.vector.tensor_copy(gT_sb[:, k, :], gtp)
            G0_ps = psA.tile([N_CLS, nshard], F32, tag="z")
            for k in range(nkt):
                for h in range(nh):
                    nc.tensor.matmul(
                        G0_ps[:, 512 * h:512 * h + ncw[h]],
                        gT_sb[:, k, :],
                        fnTs[:, k, 512 * h:512 * h + ncw[h]],
                        start=(k == 0), stop=(k == nkt - 1))
            G0_sb = tailp.tile([N_CLS, nshard], F32)
            nc.scalar.copy(G0_sb, G0_ps)

            # ---- main loop over j tiles ----
            R1_ps = psAcc.tile([N_CLS, nshard], F32, tag="R1")
            Q1_ps = psAcc.tile([N_CLS, nshard], F32, tag="Q1")
            for jt in range(njt):
                zt = psA.tile([128, nshard], F32, tag="z")
                for k in range(nkt):
                    for h in range(nh):
                        nc.tensor.matmul(
                            zt[:, 512 * h:512 * h + ncw[h]],
                            fnT[:, k, jt * 128:(jt + 1) * 128],
                            fnTs[:, k, 512 * h:512 * h + ncw[h]],
                            start=(k == 0), stop=(k == nkt - 1))
                Et = main.tile([128, nshard], BF16, tag="E")
                nc.scalar.activation(Et, zt, mybir.ActivationFunctionType.Exp,
                                     bias=negit_b, scale=INV_T)
                yt = main.tile([128, nshard], BF16, tag="y")
                nc.vector.tensor_scalar(out=yt, in0=zt, scalar1=INV_T,
                                        scalar2=-INV_T,
                                        op0=mybir.AluOpType.mult,
                                        op1=mybir.AluOpType.add)
                # zero the diagonal entries living in this j-tile
                w0 = win * jt
                nc.gpsimd.tensor_mul(Et[:, w0:w0 + win], Et[:, w0:w0 + win],
                                     mask_sb)
                yEt = main.tile([128, nshard], BF16, tag="yE")
                nc.gpsimd.tensor_mul(yEt, yt, Et)
                for h in range(nh):
                    sl = slice(512 * h, 512 * h + ncw[h])
                    nc.tensor.matmul(R1_ps[:, sl], oh_sb[:, jt, :], Et[:, sl],
                                     start=(jt == 0), stop=(jt == njt - 1))
                    nc.tensor.matmul(Q1_ps[:, sl], oh_sb[:, jt, :], yEt[:, sl],
                                     start=(jt == 0), stop=(jt == njt - 1))

            # ---- tail: per-i assembly, then scalar partial ----
            R1_sb = tailp.tile([N_CLS, nshard], F32)
            nc.scalar.copy(R1_sb, R1_ps)
            Q1_sb = tailp.tile([N_CLS, nshard], F32)
            nc.scalar.copy(Q1_sb, Q1_ps)

            # catm fields: 0 = R1*ohsel, 1 = Q1*ohsel, 2 = G0*ohsel, 3 = R1 (-> S)
            nit = nshard // 128
            catm = tailp.tile([N_CLS, 4, nshard], F32)
            nc.vector.tensor_mul(catm[:, 0, :], R1_sb, ohsel_sb)
            nc.vector.tensor_mul(catm[:, 1, :], Q1_sb, ohsel_sb)
            nc.vector.tensor_mul(catm[:, 2, :], G0_sb, ohsel_sb)
            nc.vector.tensor_copy(catm[:, 3, :], R1_sb)
            sel_sb = tailp.tile([1, 4 * nshard], F32)
            cat2d = catm.rearrange("p a b -> p (a b)")
            for h in range((4 * nshard + 511) // 512):
                w = min(512, 4 * nshard - 512 * h)
                sl = slice(512 * h, 512 * h + w)
                selp = psA.tile([1, 512], F32, tag="z")
                nc.tensor.matmul(selp[:, 0:w], ones9, cat2d[:, sl])
                nc.scalar.copy(sel_sb[:, sl], selp[:, 0:w])
            # redistribute to [i-on-partitions]: selT[p, f, t] = sel[f*ns + 128t + p]
            selT = tailp.tile([128, 4, nit], F32)
            for f in range(4):
                for t in range(nit):
                    nc.sync.dma_start(
                        selT[:, f, t:t + 1],
                        sel_sb[:, f * nshard + 128 * t:f * nshard + 128 * (t + 1)]
                        .rearrange("o (p u) -> o p u", u=1))
            R1s = selT[:, 0, :]
            Q1s = selT[:, 1, :]
            G0s = selT[:, 2, :]
            S = selT[:, 3, :]
            wv_pt = wvn_sb[:, 0, :]
            npos_pt = wvn_sb[:, 1, :]

            logS = tailp.tile([128, nit], F32)
            nc.scalar.activation(logS, S, mybir.ActivationFunctionType.Ln,
                                 bias=zero_b)
            invS = tailp.tile([128, nit], F32)
            nc.vector.reciprocal(invS, S)

            t1 = tailp.tile([128, nit], F32)
            nc.vector.tensor_mul(t1, logS, R1s)
            t2 = tailp.tile([128, nit], F32)
            nc.vector.tensor_sub(t2, Q1s, t1)
            U1 = tailp.tile([128, nit], F32)
            nc.vector.tensor_mul(U1, t2, invS)

            t3 = tailp.tile([128, nit], F32)
            nc.vector.tensor_sub(t3, G0s, npos_pt)
            t4 = tailp.tile([128, nit], F32)
            nc.vector.tensor_scalar(out=t4, in0=t3, scalar1=-1.0,
                                    scalar2=INV_T,
                                    op0=mybir.AluOpType.add,
                                    op1=mybir.AluOpType.mult)
            t5 = tailp.tile([128, nit], F32)
            nc.vector.tensor_mul(t5, npos_pt, logS)
            T0 = tailp.tile([128, nit], F32)
            nc.vector.tensor_sub(T0, t4, t5)

            row = tailp.tile([128, nit], F32)
            nc.vector.scalar_tensor_tensor(
                out=row, in0=U1, scalar=-2.0, in1=T0,
                op0=mybir.AluOpType.mult, op1=mybir.AluOpType.add)
            per = tailp.tile([128, nit], F32)
            nc.vector.tensor_mul(per, row, wv_pt)
            redp = tailp.tile([128, 1], F32)
            nc.vector.reduce_sum(redp, per, axis=mybir.AxisListType.X)
            if debug_out:
                nc.sync.dma_start(dbg_sel[:], selT)
                nc.sync.dma_start(dbg_R1[:], R1_sb)
                nc.sync.dma_start(dbg_G0[:], G0_sb)
            fin_ps = psA.tile([1, 1], F32, tag="z")
            nc.tensor.matmul(fin_ps, ones128, redp)
            red = tailp.tile([1, 1], F32)
            nc.scalar.copy(red, fin_ps)
            nc.sync.dma_start(out[:], red)

    nc.compile()
    return nc


def make_inputs(features, labels, class_weights, n_cores=N_CORES):
    """Host-side input prep: one-hot encodings, per-core shards + masks."""
    n, d = features.shape
    njt = n // 128
    win = 128 // n_cores
    labels = np.asarray(labels).astype(np.int64)
    cw = np.asarray(class_weights, dtype=np.float64)

    counts = np.bincount(labels, minlength=N_CLS).astype(np.float64)
    npos = counts[labels] - 1.0
    w = cw[labels]
    wv = np.where(npos > 0, w / np.maximum(npos, 1.0), 0.0)

    OH = (labels[:, None] == np.arange(N_CLS)[None, :])
    oh9 = OH.astype(ml_dtypes.bfloat16).reshape(njt, 128, N_CLS)

    feats_f32 = np.ascontiguousarray(features, dtype=np.float32)

    in_maps = []
    for r in range(n_cores):
        idx = np.arange(r, n, n_cores)
        m16 = np.ones((128, win), np.float32)
        m16[np.arange(win) * n_cores + r, np.arange(win)] = 0.0
        in_maps.append({
            "feats": feats_f32,
            "fshard": np.ascontiguousarray(feats_f32[idx]),
            "oh9": oh9,
            "mask16": m16.astype(ml_dtypes.bfloat16),
            "ohsel": np.ascontiguousarray(
                OH[idx].T.astype(ml_dtypes.bfloat16)),
            "wvn": np.ascontiguousarray(
                np.stack([wv[idx], npos[idx]])      # [2, nshard]
                .reshape(2, len(idx) // 128, 128)   # [2, t, p]
                .transpose(2, 0, 1).astype(np.float32)),
        })
    return in_maps


_NC_CACHE = {}


def kernel(features, labels, class_weights):
    key = features.shape
    if key not in _NC_CACHE:
        _NC_CACHE[key] = build_nc(features.shape[0], N_CORES, features.shape[1])
    nc = _NC_CACHE[key]
    in_maps = make_inputs(features, labels, class_weights)
    res = run_bass_kernel_spmd(nc, in_maps, core_ids=list(range(N_CORES)))
    total = sum(float(r["partial"][0, 0]) for r in res.results)
    return np.float32(-total / features.shape[0])


# revision 16
# speedup vs baseline: 1.4945x; 1.4945x over previous
"""Trainium2 Bass kernel for nn_CBContrastiveLoss (class-balanced focal contrastive loss).

Strategy (8-core SPMD, one compiled NEFF, per-core differences only via inputs):
  - Interleaved data-parallel sharding over samples i: core r owns rows i = r::8.
  - On each core, compute sim TRANSPOSED: tiles [j=128 partitions, i=1024 free],
    so per-row(i) reductions over j become PE matmuls with a one-hot lhsT.
  - Focal loss decomposition (validated vs reference, rel err < 1e-6):
      per_pair = (y - logS)(1 - p)^2,  y = (dot-1)/T,  p = exp(y - logS)
      sum_pos per_pair = T0 - 2*U1 + U2 (U2 dropped, ~3e-7 rel)
      T0 = (fn_i . g_c - 1 - npos)/T - npos*logS   (analytic via class sums g_c)
      U1 = (Q1 - logS*R1)/S
      R1 = sum_pos E, Q1 = sum_pos y*E, S = sum_{j != i} E = sum_c R1[c]
  - Diagonal E_ii zeroed exactly via a tiny per-core [128,16] mask input
    (interleaved sharding makes the diag position core-independent per j-tile).
"""

import numpy as np
import ml_dtypes

import concourse.bass as bass
import concourse.bacc as bacc
import concourse.tile as tile
from concourse import mybir
from concourse.bass_utils import run_bass_kernel_spmd
from concourse.masks import make_identity
from concourse import bass_isa

F32 = mybir.dt.float32
BF16 = mybir.dt.bfloat16

TEMP = 0.07
INV_T = 1.0 / TEMP

N_TOTAL = 8192
D = 512
N_CORES = 8
N_CLS = 9


def build_nc(n_total=N_TOTAL, n_cores=N_CORES, d=D, debug_out=False):
    nshard = n_total // n_cores          # i per core (free dim)
    njt = n_total // 128                 # j tiles
    nkt = d // 128                       # contraction tiles
    win = 128 // n_cores                 # diag window cols per j-tile (16)
    nh = (nshard + 511) // 512           # number of 512-wide N chunks
    ncw = [min(512, nshard - 512 * h) for h in range(nh)]

    nc = bacc.Bacc("TRN2")

    feats = nc.dram_tensor("feats", [n_total, d], F32, kind="ExternalInput")
    fshard = nc.dram_tensor("fshard", [nshard, d], F32, kind="ExternalInput")
    oh9 = nc.dram_tensor("oh9", [njt, 128, N_CLS], BF16, kind="ExternalInput")
    mask16 = nc.dram_tensor("mask16", [128, win], BF16, kind="ExternalInput")
    ohsel = nc.dram_tensor("ohsel", [N_CLS, nshard], BF16, kind="ExternalInput")
    wvn = nc.dram_tensor("wvn", [128, 2, nshard // 128], F32, kind="ExternalInput")
    out = nc.dram_tensor("partial", [1, 1], F32, kind="ExternalOutput")
    fnb_dram = nc.dram_tensor("fnb_scratch", [n_total, d], BF16)
    fnbs_dram = nc.dram_tensor("fnbs_scratch", [nshard, d], BF16)
    if debug_out:
        dbg_sel = nc.dram_tensor("dbg_sel", [128, 4, nshard // 128], F32,
                                 kind="ExternalOutput")
        dbg_R1 = nc.dram_tensor("dbg_R1", [N_CLS, nshard], F32,
                                kind="ExternalOutput")
        dbg_G0 = nc.dram_tensor("dbg_G0", [N_CLS, nshard], F32,
                                kind="ExternalOutput")

    nst = nshard // 128                  # shard row tiles

    with tile.TileContext(nc) as tc:
        with (
            tc.tile_pool(name="consts", bufs=1) as consts,
            tc.tile_pool(name="fnt", bufs=1) as fnt_pool,
            tc.tile_pool(name="pre", bufs=12) as pre,
            tc.tile_pool(name="pre2", bufs=4) as pre2,
            tc.tile_pool(name="grp", bufs=3) as grp,
            tc.tile_pool(name="main", bufs=3) as main,
            tc.tile_pool(name="tail", bufs=1) as tailp,
            tc.tile_pool(name="psA", bufs=2, space="PSUM") as psA,
            tc.tile_pool(name="psAcc", bufs=1, space="PSUM") as psAcc,
        ):
            # ---- constants ----
            oh_sb = consts.tile([128, njt, N_CLS], BF16)
            nc.sync.dma_start(oh_sb, oh9[:].rearrange("t p c -> p t c"))
            mask_sb = consts.tile([128, win], BF16)
            nc.sync.dma_start(mask_sb, mask16[:])
            ohsel_sb = consts.tile([N_CLS, nshard], BF16)
            nc.sync.dma_start(ohsel_sb, ohsel[:])
            wvn_sb = consts.tile([128, 2, nshard // 128], F32)
            nc.sync.dma_start(wvn_sb, wvn[:])
            ident = consts.tile([128, 128], F32)
            make_identity(nc, ident)
            ones9 = consts.tile([N_CLS, 1], F32)
            nc.vector.memset(ones9, 1.0)
            ones128 = consts.tile([128, 1], F32)
            nc.vector.memset(ones128, 1.0)
            zero_b = consts.tile([128, 1], F32)
            nc.vector.memset(zero_b, 0.0)
            negit_b = consts.tile([128, 1], F32)
            nc.vector.memset(negit_b, -INV_T)
            # warmup activation: absorbs the ACT table-load wait (walrus
            # attaches it to the first ACTIVATE, which then allows only one
            # user wait)
            warm = consts.tile([128, 1], F32)
            nc.scalar.activation(warm, zero_b,
                                 mybir.ActivationFunctionType.Exp,
                                 bias=zero_b)

            fnT = fnt_pool.tile([128, nkt, n_total], BF16)   # full, transposed
            fnTs = fnt_pool.tile([128, nkt, nshard], BF16)   # shard, transposed

            g_ps = psAcc.tile([N_CLS, d], F32, tag="R1")

            # ---- preamble: normalize features, build fnT + class sums g ----
            def norm_tiles(src_dram, ntiles, dst_dram, with_g):
                group = 8
                for t0 in range(0, ntiles, group):
                    gn = min(group, ntiles - t0)
                    n2g = grp.tile([128, group], F32, tag="n2")
                    rng = grp.tile([128, group], F32, tag="rn")
                    fts = []
                    for t in range(t0, t0 + gn):
                        ft = pre.tile([128, d], F32, tag="ft")
                        nc.sync.dma_start(ft, src_dram[t * 128:(t + 1) * 128, :])
                        sq = pre2.tile([128, d], F32, tag="sq")
                        nc.vector.scalar_tensor_tensor(
                            out=sq, in0=ft, scalar=1.0, in1=ft,
                            op0=mybir.AluOpType.mult,
                            op1=mybir.AluOpType.mult,
                            accum_out=n2g[:, t - t0:t - t0 + 1],
                        )
                        fts.append(ft)
                    # rn = exp(-0.5 * ln(n2))  (avoids inaccurate Rsqrt table)
                    nc.scalar.activation(rng[:, 0:gn], n2g[:, 0:gn],
                                         mybir.ActivationFunctionType.Ln,
                                         bias=zero_b)
                    nc.scalar.activation(rng[:, 0:gn], rng[:, 0:gn],
                                         mybir.ActivationFunctionType.Exp,
                                         bias=zero_b, scale=-0.5)
                    for t in range(t0, t0 + gn):
                        ft = fts[t - t0]
                        fnb = pre2.tile([128, d], BF16, tag="fnb")
                        nc.vector.tensor_scalar_mul(
                            out=fnb, in0=ft, scalar1=rng[:, t - t0:t - t0 + 1])
                        if with_g:
                            nc.tensor.matmul(
                                g_ps, oh_sb[:, t, :], fnb,
                                start=(t == 0), stop=(t == ntiles - 1))
                        nc.sync.dma_start(
                            dst_dram[t * 128:(t + 1) * 128, :], fnb)

            norm_tiles(feats, njt, fnb_dram, with_g=True)
            norm_tiles(fshard, nst, fnbs_dram, with_g=False)

            # transpose via xbar DMA from DRAM in big strips; alternate the
            # two HWDGE issuing engines (sync / scalar)
            eng = [nc.sync, nc.scalar]
            strip = min(1024, nshard)
            ei = 0
            for k in range(nkt):
                for s0 in range(0, n_total, strip):
                    eng[ei % 2].dma_start_transpose(
                        fnT[:, k, s0:s0 + strip],
                        fnb_dram[s0:s0 + strip, k * 128:(k + 1) * 128])
                    ei += 1
                for s0 in range(0, nshard, strip):
                    eng[ei % 2].dma_start_transpose(
                        fnTs[:, k, s0:s0 + strip],
                        fnbs_dram[s0:s0 + strip, k * 128:(k + 1) * 128])
                    ei += 1

            # ---- g -> gT (bf16) ; G0[c,i] = fn_i . g_c ----
            g_sb = tailp.tile([N_CLS, d], F32)
            nc.scalar.copy(g_sb, g_ps)
            gT_sb = tailp.tile([128, nkt, N_CLS], BF16)
            for k in range(nkt):
                gtp = psA.tile([128, N_CLS], F32, tag="z")
                nc.tensor.transpose(gtp, g_sb[0:N_CLS, k * 128:(k + 1) * 128],
                                    ident[0:N_CLS, 0:N_CLS])
                nc.vector.tensor_copy(gT_sb[:, k, :], gtp)
            G0_ps = psA.tile([N_CLS, nshard], F32, tag="z")
            for k in range(nkt):
                for h in range(nh):
                    nc.tensor.matmul(
                        G0_ps[:, 512 * h:512 * h + ncw[h]],
                        gT_sb[:, k, :],
                        fnTs[:, k, 512 * h:512 * h + ncw[h]],
                        start=(k == 0), stop=(k == nkt - 1))
            G0_sb = tailp.tile([N_CLS, nshard], F32)
            nc.scalar.copy(G0_sb, G0_ps)

            # ---- main loop over j tiles ----
            R1_ps = psAcc.tile([N_CLS, nshard], F32, tag="R1")
            Q1_ps = psAcc.tile([N_CLS, nshard], F32, tag="Q1")
            for jt in range(njt):
                zt = psA.tile([128, nshard], F32, tag="z")
                for k in range(nkt):
                    for h in range(nh):
                        nc.tensor.matmul(
                            zt[:, 512 * h:512 * h + ncw[h]],
                            fnT[:, k, jt * 128:(jt + 1) * 128],
                            fnTs[:, k, 512 * h:512 * h + ncw[h]],
                            start=(k == 0), stop=(k == nkt - 1))
                Et = main.tile([128, nshard], BF16, tag="E")
                nc.scalar.activation(Et, zt, mybir.ActivationFunctionType.Exp,
                                     bias=negit_b, scale=INV_T)
                yt = main.tile([128, nshard], BF16, tag="y")
                nc.vector.tensor_scalar(out=yt, in0=zt, scalar1=INV_T,
                                        scalar2=-INV_T,
                                        op0=mybir.AluOpType.mult,
                                        op1=mybir.AluOpType.add)
                # zero the diagonal entries living in this j-tile
                w0 = win * jt
                nc.gpsimd.tensor_mul(Et[:, w0:w0 + win], Et[:, w0:w0 + win],
                                     mask_sb)
                yEt = main.tile([128, nshard], BF16, tag="yE")
                nc.gpsimd.tensor_mul(yEt, yt, Et)
                for h in range(nh):
                    sl = slice(512 * h, 512 * h + ncw[h])
                    nc.tensor.matmul(R1_ps[:, sl], oh_sb[:, jt, :], Et[:, sl],
                                     start=(jt == 0), stop=(jt == njt - 1))
                    nc.tensor.matmul(Q1_ps[:, sl], oh_sb[:, jt, :], yEt[:, sl],
                                     start=(jt == 0), stop=(jt == njt - 1))

            # ---- tail: per-i assembly, then scalar partial ----
            R1_sb = tailp.tile([N_CLS, nshard], F32)
            nc.scalar.copy(R1_sb, R1_ps)
            Q1_sb = tailp.tile([N_CLS, nshard], F32)
            nc.scalar.copy(Q1_sb, Q1_ps)

            # catm fields: 0 = R1*ohsel, 1 = Q1*ohsel, 2 = G0*ohsel, 3 = R1 (-> S)
            nit = nshard // 128
            catm = tailp.tile([N_CLS, 4, nshard], F32)
            nc.vector.tensor_mul(catm[:, 0, :], R1_sb, ohsel_sb)
            nc.vector.tensor_mul(catm[:, 1, :], Q1_sb, ohsel_sb)
            nc.vector.tensor_mul(catm[:, 2, :], G0_sb, ohsel_sb)
            nc.vector.tensor_copy(catm[:, 3, :], R1_sb)
            sel_sb = tailp.tile([1, 4 * nshard], F32)
            cat2d = catm.rearrange("p a b -> p (a b)")
            for h in range((4 * nshard + 511) // 512):
                w = min(512, 4 * nshard - 512 * h)
                sl = slice(512 * h, 512 * h + w)
                selp = psA.tile([1, 512], F32, tag="z")
                nc.tensor.matmul(selp[:, 0:w], ones9, cat2d[:, sl])
                nc.scalar.copy(sel_sb[:, sl], selp[:, 0:w])
            # redistribute to [i-on-partitions]: selT[p, f, t] = sel[f*ns + 128t + p]
            selT = tailp.tile([128, 4, nit], F32)
            for f in range(4):
                for t in range(nit):
                    nc.sync.dma_start(
                        selT[:, f, t:t + 1],
                        sel_sb[:, f * nshard + 128 * t:f * nshard + 128 * (t + 1)]
                        .rearrange("o (p u) -> o p u", u=1))
            R1s = selT[:, 0, :]
            Q1s = selT[:, 1, :]
            G0s = selT[:, 2, :]
            S = selT[:, 3, :]
            wv_pt = wvn_sb[:, 0, :]
            npos_pt = wvn_sb[:, 1, :]

            logS = tailp.tile([128, nit], F32)
            nc.scalar.activation(logS, S, mybir.ActivationFunctionType.Ln,
                                 bias=zero_b)
            invS = tailp.tile([128, nit], F32)
            nc.vector.reciprocal(invS, S)

            t1 = tailp.tile([128, nit], F32)
            nc.vector.tensor_mul(t1, logS, R1s)
            t2 = tailp.tile([128, nit], F32)
            nc.vector.tensor_sub(t2, Q1s, t1)
            U1 = tailp.tile([128, nit], F32)
            nc.vector.tensor_mul(U1, t2, invS)

            t3 = tailp.tile([128, nit], F32)
            nc.vector.tensor_sub(t3, G0s, npos_pt)
            t4 = tailp.tile([128, nit], F32)
            nc.vector.tensor_scalar(out=t4, in0=t3, scalar1=-1.0,
                                    scalar2=INV_T,
                                    op0=mybir.AluOpType.add,
                                    op1=mybir.AluOpType.mult)
            t5 = tailp.tile([128, nit], F32)
            nc.vector.tensor_mul(t5, npos_pt, logS)
            T0 = tailp.tile([128, nit], F32)
            nc.vector.tensor_sub(T0, t4, t5)

            row = tailp.tile([128, nit], F32)
            nc.vector.scalar_tensor_tensor(
                out=row, in0=U1, scalar=-2.0, in1=T0,
                op0=mybir.AluOpType.mult, op1=mybir.AluOpType.add)
            per = tailp.tile([128, nit], F32)
            nc.vector.tensor_mul(per, row, wv_pt)
            redp = tailp.tile([128, 1], F32)
            nc.vector.reduce_sum(redp, per, axis=mybir.AxisListType.X)
            if debug_out:
                nc.sync.dma_start(dbg_sel[:], selT)
                nc.sync.dma_start(dbg_R1[:], R1_sb)
                nc.sync.dma_start(dbg_G0[:], G0_sb)
            fin_ps = psA.tile([1, 1], F32, tag="z")
            nc.tensor.matmul(fin_ps, ones128, redp)
            red = tailp.tile([1, 1], F32)
            nc.scalar.copy(red, fin_ps)
            nc.sync.dma_start(out[:], red)

    nc.compile()
    return nc


def make_inputs(features, labels, class_weights, n_cores=N_CORES):
    """Host-side input prep: one-hot encodings, per-core shards + masks."""
    n, d = features.shape
    njt = n // 128
    win = 128 // n_cores
    labels = np.asarray(labels).astype(np.int64)
    cw = np.asarray(class_weights, dtype=np.float64)

    counts = np.bincount(labels, minlength=N_CLS).astype(np.float64)
    npos = counts[labels] - 1.0
    w = cw[labels]
    wv = np.where(npos > 0, w / np.maximum(npos, 1.0), 0.0)

    OH = (labels[:, None] == np.arange(N_CLS)[None, :])
    oh9 = OH.astype(ml_dtypes.bfloat16).reshape(njt, 128, N_CLS)

    feats_f32 = np.ascontiguousarray(features, dtype=np.float32)

    in_maps = []
    for r in range(n_cores):
        idx = np.arange(r, n, n_cores)
        m16 = np.ones((128, win), np.float32)
        m16[np.arange(win) * n_cores + r, np.arange(win)] = 0.0
        in_maps.append({
            "feats": feats_f32,
            "fshard": np.ascontiguousarray(feats_f32[idx]),
            "oh9": oh9,
            "mask16": m16.astype(ml_dtypes.bfloat16),
            "ohsel": np.ascontiguousarray(
                OH[idx].T.astype(ml_dtypes.bfloat16)),
            "wvn": np.ascontiguousarray(
                np.stack([wv[idx], npos[idx]])      # [2, nshard]
                .reshape(2, len(idx) // 128, 128)   # [2, t, p]
                .transpose(2, 0, 1).astype(np.float32)),
        })
    return in_maps


_NC_CACHE = {}


def kernel(features, labels, class_weights):
    key = features.shape
    if key not in _NC_CACHE:
        _NC_CACHE[key] = build_nc(features.shape[0], N_CORES, features.shape[1])
    nc = _NC_CACHE[key]
    in_maps = make_inputs(features, labels, class_weights)
    res = run_bass_kernel_spmd(nc, in_maps, core_ids=list(range(N_CORES)))
    total = sum(float(r["partial"][0, 0]) for r in res.results)
    return np.float32(-total / features.shape[0])
